# revision 11
# baseline (speedup 1.0000x reference)
"""Trainium2 Bass kernel for the CAM (channel-attention) module.

Reference computation (per batch b):
    energy  = x @ x.T                      # [C, C], contraction over N
    att     = softmax(rowmax(energy) - energy, axis=-1)
            = softmin of energy rows       # (the rowmax cancels in softmax)
    out     = gamma * (att @ x) + x

Shapes: x [B=16, C=64, N=65536] f32, gamma [1] f32.

Sharding: data-parallel over batch across 8 cores (2 batches per core).

Per-core layout trick: each batch's [64, 65536] slab is viewed as
[128, 32768] with partition p = h*64 + c  (h = which half of N).  This keeps
all 128 partitions busy.  The energy then splits as E = E_h0 + E_h1 where
each half is a [64, 64] Gram matrix over its half of N, and the apply phase
uses a 128x128 block-diagonal attention matrix.

Pipeline per batch:
  pass 1: stream fp32 chunks in; cast to bf16 (ACT); transpose 128x128 tiles
          via plain matmuls against the identity (stationary = x tile, moving
          = I); copy transposed tiles PSUM->SBUF as bf16 (DVE/ACT); Gram
          matmuls accumulate E_h0/E_h1 in PSUM.
  softmax: E = E_h0 + E_h1 -> softmin rows -> att * gamma -> transpose via
          matmul (col-tiled to build the block-diagonal lhsT).
  pass 2: re-cast resident fp32 chunks to bf16, matmul against the
          block-diag attention, add x in fp32 (DVE, in place), DMA out.

The fp32 x stays resident in SBUF between pass 1 and pass 2 (16 MB/batch),
so HBM traffic is the minimum 32 MB read + 32 MB write per core.
"""

import numpy as np
import ml_dtypes

import concourse.bass as bass
import concourse.bacc as bacc
import concourse.mybir as mybir
import concourse.tile as tile

F32 = mybir.dt.float32
F32R = mybir.dt.float32r
BF16 = mybir.dt.bfloat16

# Full-problem constants (hardcoded per the grading contract).
B_FULL = 16
C = 64
N_FULL = 65536
N_CORES = 8
B_CORE = B_FULL // N_CORES  # 2 batches per core
H = 2                       # N-halves packed into partitions
P = H * C                   # 128 partitions
NV_FULL = N_FULL // H       # 32768 view columns per batch

KT = 128     # transpose/Gram K-tile (partition-dim contraction size)
GROUP = 512  # transposed columns per PSUM bank / per PSUM->SBUF copy
OUT_TILE = 512  # pass-2 matmul free size (one PSUM bank of fp32)


def build_nc(b_core=B_CORE, nv=NV_FULL, chunk=2048, x32_bufs=20):
    """Build the per-core Bass module. x input is host-packed [b, 128, nv]."""
    assert chunk % GROUP == 0 and GROUP % KT == 0 and nv % chunk == 0
    assert chunk % OUT_TILE == 0

    nc = bacc.Bacc("TRN2", target_bir_lowering=False)
    x_d = nc.dram_tensor("x", [b_core, P, nv], F32, kind="ExternalInput")
    ident_d = nc.dram_tensor("ident", [P, P], BF16, kind="ExternalInput")
    ident64_d = nc.dram_tensor("ident64", [C, C], F32, kind="ExternalInput")
    gamma_d = nc.dram_tensor("gamma64", [C, 1], F32, kind="ExternalInput")
    out_d = nc.dram_tensor("out", [b_core, P, nv], F32, kind="ExternalOutput")

    nchunks = nv // chunk
    kt_total = nv // KT

    with tile.TileContext(nc) as tc:
        with (
            tc.tile_pool(name="consts", bufs=1) as consts,
            tc.tile_pool(name="x32", bufs=x32_bufs) as x32_pool,
            tc.tile_pool(name="xb16", bufs=3) as xb16_pool,
            tc.tile_pool(name="xtg", bufs=4) as xtg_pool,
            tc.tile_pool(name="small", bufs=2) as small,
            tc.tile_pool(name="psT", bufs=2, space=bass.MemorySpace.PSUM) as psT_pool,
            tc.tile_pool(name="psE", bufs=1, space=bass.MemorySpace.PSUM) as psE_pool,
            tc.tile_pool(name="psA", bufs=1, space=bass.MemorySpace.PSUM) as psA_pool,
            tc.tile_pool(name="psO", bufs=2, space=bass.MemorySpace.PSUM) as psO_pool,
        ):
            ident_sb = consts.tile([P, P], BF16, tag="ident")
            nc.sync.dma_start(ident_sb[:], ident_d[:])
            ident64_sb = consts.tile([C, C], F32, tag="ident64")
            nc.sync.dma_start(ident64_sb[:], ident64_d[:])
            gam = consts.tile([C, 1], F32, tag="gam")
            nc.sync.dma_start(gam[:], gamma_d[:])

            for b in range(b_core):
                xv = x_d[b]
                ov = out_d[b]

                psE0 = psE_pool.tile([C, C], F32, tag="psE0")
                psE1 = psE_pool.tile([C, C], F32, tag="psE1")

                # ---- pass 1: load, cast, transpose, Gram-accumulate ----
                xts = []
                kti = 0
                for ci in range(nchunks):
                    xt = x32_pool.tile([P, chunk], F32, tag="x32")
                    nc.sync.dma_start(xt[:], xv[:, ci * chunk:(ci + 1) * chunk])
                    xts.append(xt)

                    xb = xb16_pool.tile([P, chunk], BF16, tag="xb16")
                    nc.scalar.copy(xb[:], xt[:])

                    for g in range(chunk // GROUP):
                        psT = psT_pool.tile([P, GROUP], F32, tag="psT")
                        for k in range(GROUP // KT):
                            col = g * GROUP + k * KT
                            nc.tensor.matmul(
                                psT[:, k * KT:(k + 1) * KT],
                                xb[:, col:col + KT],
                                ident_sb[:],
                                start=True, stop=True,
                            )
                        xtg = xtg_pool.tile([P, GROUP], BF16, tag="xtg")
                        if g % 2 == 0:
                            nc.vector.tensor_copy(xtg[:], psT[:])
                        else:
                            nc.scalar.copy(xtg[:], psT[:])
                        for k in range(GROUP // KT):
                            st = kti == 0
                            sp = kti == kt_total - 1
                            t0 = xtg[:, k * KT:k * KT + C]
                            t1 = xtg[:, k * KT + C:k * KT + 2 * C]
                            nc.tensor.matmul(psE0[:], t0, t0, start=st, stop=sp,
                                             skip_group_check=True)
                            nc.tensor.matmul(psE1[:], t1, t1, start=st, stop=sp,
                                             skip_group_check=True)
                            kti += 1

                # ---- softmin + gamma fold + block-diag lhsT ----
                e1sb = small.tile([C, C], F32, tag="e1sb")
                nc.scalar.copy(e1sb[:], psE1[:])
                E = small.tile([C, C], F32, tag="E")
                nc.vector.tensor_add(E[:], psE0[:], e1sb[:])

                mn = small.tile([C, 1], F32, tag="mn")
                nc.vector.tensor_reduce(mn[:], E[:], axis=mybir.AxisListType.X,
                                        op=mybir.AluOpType.min)
                pexp = small.tile([C, C], F32, tag="pexp")
                ssum = small.tile([C, 1], F32, tag="ssum")
                nc.scalar.activation(pexp[:], E[:],
                                     mybir.ActivationFunctionType.Exp,
                                     bias=mn[:], scale=-1.0, accum_out=ssum[:])
                rec = small.tile([C, 1], F32, tag="rec")
                nc.vector.reciprocal(rec[:], ssum[:])
                rg = small.tile([C, 1], F32, tag="rg")
                nc.vector.tensor_mul(rg[:], rec[:], gam[:])
                attg = small.tile([C, C], F32, tag="attg")
                nc.vector.tensor_scalar_mul(attg[:], pexp[:], rg[:])

                # block-diag lhsT in fp32 (consumed as f32r by pass 2)
                psA = psA_pool.tile([P, P], F32, tag="psA")
                nc.vector.memset(psA[0:C, C:P], 0.0)
                nc.vector.memset(psA[C:P, 0:C], 0.0)
                nc.tensor.matmul(psA[0:C, 0:C], attg[:], ident64_sb[:],
                                 start=True, stop=True)
                nc.tensor.matmul(psA[C:P, C:P], attg[:], ident64_sb[:],
                                 start=True, stop=True)
                bd = small.tile([P, P], F32, tag="bd")
                nc.vector.tensor_copy(bd[:], psA[:])

                # ---- pass 2: apply attention (f32r reads x directly), add x,
                # store ----
                for ci in range(nchunks):
                    xt = xts[ci]
                    for s in range(chunk // OUT_TILE):
                        sl = slice(s * OUT_TILE, (s + 1) * OUT_TILE)
                        psO = psO_pool.tile([P, OUT_TILE], F32, tag="psO")
                        nc.tensor.matmul(psO[:], bd[:].bitcast(F32R),
                                         xt[:, sl].bitcast(F32R),
                                         start=True, stop=True)
                        nc.vector.tensor_add(xt[:, sl], xt[:, sl], psO[:])
                    nc.scalar.dma_start(ov[:, ci * chunk:(ci + 1) * chunk], xt[:])

    nc.compile()
    return nc


def pack_inputs(x_core, gamma):
    """x_core [b, C, N] f32 -> h-major view [b, 128, N//2], plus constants."""
    b = x_core.shape[0]
    n = x_core.shape[2]
    xv = np.ascontiguousarray(
        x_core.reshape(b, C, H, n // H).transpose(0, 2, 1, 3)
    ).reshape(b, P, n // H)
    ident = np.eye(P, dtype=ml_dtypes.bfloat16)
    ident64 = np.eye(C, dtype=np.float32)
    g64 = np.broadcast_to(np.asarray(gamma, np.float32).reshape(1, 1), (C, 1))
    return {
        "x": xv,
        "ident": ident,
        "ident64": ident64,
        "gamma64": np.ascontiguousarray(g64),
    }


def unpack_output(out_view, n):
    """[b, 128, n//2] h-major view -> [b, C, n]."""
    b = out_view.shape[0]
    return np.ascontiguousarray(
        out_view.reshape(b, H, C, n // H).transpose(0, 2, 1, 3)
    ).reshape(b, C, n)


_NC_CACHE = {}

# Last BassKernelResults from kernel() — lets a test harness read
# exec_time_ns when run with BASS_TRACE=1.
LAST_RESULTS = None


def kernel(x, gamma):
    from concourse import bass_utils

    x = np.asarray(x, dtype=np.float32)
    gamma = np.asarray(gamma, dtype=np.float32)
    assert x.shape == (B_FULL, C, N_FULL), x.shape

    key = "full"
    if key not in _NC_CACHE:
        _NC_CACHE[key] = build_nc()
    nc = _NC_CACHE[key]

    in_maps = []
    for core in range(N_CORES):
        x_core = x[core * B_CORE:(core + 1) * B_CORE]
        in_maps.append(pack_inputs(x_core, gamma))

    res = bass_utils.run_bass_kernel_spmd(
        nc, in_maps, core_ids=list(range(N_CORES))
    )
    global LAST_RESULTS
    LAST_RESULTS = res
    outs = [unpack_output(r["out"], N_FULL) for r in res.results]
    return np.concatenate(outs, axis=0)
